# revision 30
# baseline (speedup 1.0000x reference)
"""Chamfer-distance block (EG3D ray sampler + point-cloud chamfer) on 8 trn2 cores.

Sharding: core c = 2*b + h handles batch b (of 4) and ray-half h (8192 of
16384 rays).  Host does the O(M)+O(N) prep (exact float64 ray sampler,
pred points, fp8 3-level hi/lo splits, point-pair transform, max_distance);
the device does the O(M*N) retrieval:
  - row-min over the 8192x2048 squared-distance matrix via the min-pair
    identity min(a,b) = (a+b - |a-b|)/2: the PE emits pair-sum columns
    s_j = cross_2j + cross_2j+1 and pair-diff columns d_j = cross_2j -
    cross_2j+1 (1024 of each per 128-ray tile) using fp8e4m3 3-level
    hi/lo-split matmuls in DoubleRow perf mode (24 k-rows as 12
    partitions x 2 groups).  Drain per tile: ACT reads the d-half of
    PSUM (Abs -> fp16 SBUF), DVE reads the s-half with one fused custom
    DVE op ((s - |d|)*0.5, min-accumulated per ray), so each engine
    streams exactly half the PSUM traffic.  The d-halves are split into
    single-bank PSUM tiles with their own ABS each, so banks free early
    and the PE never stalls at tile-pair boundaries (a stall resets the
    PE's p-state clock ramp and slows every matmul 2-4x).
  - masked partial sums (numerator, denominator)
Host combines the two halves of each batch: loss = num / max(den, 1).
"""

import os
import sys

import numpy as np

if "/opt/trn_rl_repo" not in sys.path:
    sys.path.insert(0, "/opt/trn_rl_repo")

import concourse.bass as bass
import concourse.bacc as bacc
import concourse.mybir as mybir
import concourse.tile as tile
from concourse.bass import ts
from concourse.masks import make_identity

F32 = mybir.dt.float32
F16 = mybir.dt.float16
FP8 = mybir.dt.float8e4
I32 = mybir.dt.int32

B = 4
RES = 128
M = RES * RES          # 16384 rays per batch
MLOC = M // 2          # 8192 rays per core
N = 2048               # points
NPAIR = N // 2         # 1024 point pairs
NT = 64                # m-tiles of 128 rays per core
NPAR = 16              # host-computed per-core scalar params
NROWS = 24             # fp8 k-rows (12 partitions x 2 DoubleRow groups)
NP8 = NROWS // 2

P_MD = 0               # params[0] = max_distance

_CACHED_NC = None
KVER = 36  # bump to bust the NEFF cache when kernel structure changes


def _register_pair_min_op():
    """Runtime-register a custom DVE op: out=(in0-in1)*imm2 with a
    min-reduce accumulator seeded from s0.  One DVE pass fuses the
    (s - |d|)*0.5 pair-min with the per-ray min reduction (the stock
    InstTensorTensorReduce crashes the exec unit on this fw)."""
    import concourse.dve_ops as dve_ops

    name = "PAIR_MIN_REDUCE_ANT"
    for op in dve_ops.OPS:
        if op.name == name:
            return op
    from concourse.dve_spec import C0, C2, Spec, Src0, Src1, _has_src1, lower, minn
    from concourse.dve_uop import DveOpSpec

    def _ref(in0, in1, c0, c1, c2):
        body = (in0.astype(np.float32) - in1) * c2
        return body, dve_ops._accum_ref(body, c0, minn, False)

    spec = Spec(body=(Src0 - Src1) * C2, accum=minn, accum_init=C0, reference=_ref)
    row = max(dve_ops._SUB_OPCODE_FOR_NAME.values()) + 1
    assert row < 0x20
    dve_ops._SUB_OPCODE_FOR_NAME[name] = row
    shas = {}
    for ver in ("v3", "v4"):
        shas[ver] = DveOpSpec(
            name=name, opcode=row, uops=lower(spec, ver=ver), rd1_en=_has_src1(spec)
        ).sha(ver)
    op = dve_ops.DveOp(name, spec, subdim=False, uops_sha=shas)
    dve_ops.OPS.append(op)
    dve_ops.CUSTOM_DVE_SPECS[name] = spec
    return op


def _patch_compiler_flags():
    """Enable walrus's ldweights dedup: consecutive matmuls sharing a
    stationary operand skip the redundant reload (PE is LDW-serialized
    otherwise)."""
    from concourse import bass_utils as _bu

    if getattr(_bu, "_ldwopt_patched", False):
        return
    _orig = _bu.run_command

    def _patched(argv, **kw):
        return _orig(argv, **kw)

    _bu.run_command = _patched
    _bu._ldwopt_patched = True


def _build_nc():
    _patch_compiler_flags()
    nc = bacc.Bacc()
    nc.dram_tensor(f"ver{KVER}", [1], F32)
    depth_d = nc.dram_tensor("depth", [MLOC], F32, kind="ExternalInput")
    rin_d = nc.dram_tensor("rin", [NP8 * 2 * N], FP8, kind="ExternalInput")
    lin_d = nc.dram_tensor("lin", [NP8 * 2 * MLOC], FP8, kind="ExternalInput")
    p2_d = nc.dram_tensor("p2in", [MLOC], F32, kind="ExternalInput")
    par_d = nc.dram_tensor("params", [NPAR], F32, kind="ExternalInput")
    out_d = nc.dram_tensor("out", [1, 2], F32, kind="ExternalOutput")

    with tile.TileContext(nc) as tc:
        _trace_kernel(tc, depth_d, rin_d, lin_d, p2_d, par_d, out_d)
    nc.finalize()
    return nc


def _trace_kernel(tc, depth_d, rin_d, lin_d, p2_d, par_d, out_d):
    nc = tc.nc
    AL = mybir.AluOpType
    ACT = mybir.ActivationFunctionType
    pair_min_op = _register_pair_min_op()

    import contextlib

    with contextlib.ExitStack() as ctx:
        singles = ctx.enter_context(tc.tile_pool(name="singles", bufs=1))
        temps = ctx.enter_context(tc.tile_pool(name="temps", bufs=2))
        psum = ctx.enter_context(tc.tile_pool(name="psum", bufs=1, space="PSUM"))
        scratchp = ctx.enter_context(tc.tile_pool(name="scratchp", bufs=4))

        # ---- load inputs (all DMAs fan out in parallel) ------------------
        par = singles.tile([128, NPAR], F32)
        nc.sync.dma_start(
            out=par,
            in_=bass.AP(tensor=par_d, offset=0, ap=[[0, 128], [1, NPAR]]),
        )

        D = singles.tile([64, RES], F32)  # depth, ray m = p*128 + f
        nc.sync.dma_start(out=D, in_=depth_d.rearrange("(p f) -> p f", f=RES))

        P2 = singles.tile([64, RES], F32)  # |pred|^2, host-computed
        nc.sync.dma_start(out=P2, in_=p2_d.rearrange("(p f) -> p f", f=RES))

        # R-side fp8 rows (host): [12, 2 groups, 2048] dup'd per quadrant.
        # Separate A/B tiles so each quadrant's matmuls start as soon as its
        # own DMAs land.
        RbufA = singles.tile([128, 2 * N], FP8)
        RbufB = singles.tile([128, 2 * N], FP8)
        qeng = [nc.sync, nc.gpsimd]
        rv_in = rin_d.rearrange("(p g f) -> p g f", g=2, f=N)
        for qi_r, (rb, base) in enumerate(((RbufA, 0), (RbufB, 64))):
            qeng[qi_r].dma_start(
                out=rb[base : base + NP8, :].rearrange("p (g f) -> p g f", g=2),
                in_=rv_in,
            )

        # L-side fp8 rows (host): [12, 2, 8192]; m-tiles 0-31 -> quadrant A,
        # 32-63 -> B.  Chunked along rays for DMA-queue overlap.
        LbufA = singles.tile([128, 2 * 4096], FP8)
        LbufB = singles.tile([128, 2 * 4096], FP8)
        lv_in = lin_d.rearrange("(p g f) -> p g f", g=2, f=MLOC)
        qi = 0
        CH = 1024
        for lb, base, lo in ((LbufA, 0, 0), (LbufB, 64, 4096)):
            for c0 in range(0, 4096, CH):
                qeng[qi % 2].dma_start(
                    out=lb[base : base + NP8, :].rearrange("p (g f) -> p g f", g=2)[
                        :, :, c0 : c0 + CH
                    ],
                    in_=lv_in[:, :, lo + c0 : lo + c0 + CH],
                )
                qi += 1

        identity = singles.tile([128, 128], F32)
        make_identity(nc, identity)

        # warm the ACT Abs table before the loop
        warm = singles.tile([1, 1], F32)
        nc.vector.memset(warm, 1.0)
        nc.scalar.activation(out=warm, in_=warm, func=ACT.Abs, bias=0.0, scale=1.0)

        # ray mask: depth < max_distance (host-computed scalar)
        mask = temps.tile([64, RES], F32)
        nc.vector.tensor_scalar(
            out=mask, in0=D, scalar1=par[:64, P_MD : P_MD + 1], scalar2=None, op0=AL.is_lt
        )
        ones64 = singles.tile([64, 1], F32)
        nc.vector.memset(ones64, 1.0)

        # ---- main loop: DoubleRow matmuls + split s/|d| drains -----------
        # d-chunks are emitted before s-chunks so each tile's psD frees
        # early (ABS starts while the s-matmuls still stream).
        rmin = singles.tile([128, NT], F32)
        DR = mybir.MatmulPerfMode.DoubleRow

        LvA = LbufA.rearrange("p (g c) -> p g c", g=2)
        LvB = LbufB.rearrange("p (g c) -> p g c", g=2)
        RvA = RbufA.rearrange("p (g c) -> p g c", g=2)
        RvB = RbufB.rearrange("p (g c) -> p g c", g=2)

        for p in range(NT // 2):
            psS_A = psum.tile([128, NPAIR], F32, tag="psSA")
            psS_B = psum.tile([128, NPAIR], F32, tag="psSB")
            psD_A0 = psum.tile([128, 512], F32, tag="psDA0")
            psD_A1 = psum.tile([128, 512], F32, tag="psDA1")
            psD_B0 = psum.tile([128, 512], F32, tag="psDB0")
            psD_B1 = psum.tile([128, 512], F32, tag="psDB1")
            for quad, nt in (
                ("A", 2), ("A", 3), ("B", 2), ("B", 3),
                ("A", 0), ("B", 0), ("A", 1), ("B", 1),
            ):
                off = (nt % 2) * 512
                if quad == "A":
                    if nt == 2:
                        dst, doff = psD_A0, 0
                    elif nt == 3:
                        dst, doff = psD_A1, 0
                    else:
                        dst, doff = psS_A, off
                    nc.tensor.matmul(
                        dst[:, doff : doff + 512],
                        lhsT=LvA[0:NP8, :, ts(p, 128)],
                        rhs=RvA[0:NP8, :, ts(nt, 512)],
                        start=True, stop=True, tile_position=(0, 0),
                        perf_mode=DR,
                    )
                else:
                    if nt == 2:
                        dst, boff = psD_B0, 0
                    elif nt == 3:
                        dst, boff = psD_B1, 0
                    else:
                        dst, boff = psS_B, off
                    nc.tensor.matmul(
                        dst[:, boff : boff + 512],
                        lhsT=LvB[64 : 64 + NP8, :, ts(p, 128)],
                        rhs=RvB[64 : 64 + NP8, :, ts(nt, 512)],
                        start=True, stop=True, tile_position=(64, 0),
                        perf_mode=DR,
                    )
            absd_A = scratchp.tile([128, NPAIR], F16, tag="absdA")
            nc.scalar.activation(out=absd_A[:, 0:512], in_=psD_A0, func=ACT.Abs, bias=0.0, scale=1.0)
            nc.scalar.activation(out=absd_A[:, 512:1024], in_=psD_A1, func=ACT.Abs, bias=0.0, scale=1.0)
            absd_B = scratchp.tile([128, NPAIR], F16, tag="absdB")
            nc.scalar.activation(out=absd_B[:, 0:512], in_=psD_B0, func=ACT.Abs, bias=0.0, scale=1.0)
            nc.scalar.activation(out=absd_B[:, 512:1024], in_=psD_B1, func=ACT.Abs, bias=0.0, scale=1.0)
            for ps_s, absd, t in ((psS_A, absd_A, p), (psS_B, absd_B, 32 + p)):
                junk = scratchp.tile([128, NPAIR], F16, tag="junk")
                nc.vector._custom_dve(
                    pair_min_op, out=junk, in0=ps_s[:, :], in1=absd[:, :],
                    s0=1e30, s1=0.0, imm2=0.5,
                    accum_out=rmin[:, t : t + 1],
                )

        # ---- final: transpose rmin back to ray layout, mask, sums --------
        rT = psum.tile([64, 128], F32, tag="psSA")
        nc.tensor.transpose(rT, rmin, identity)

        mind2 = temps.tile([64, RES], F32)
        nc.vector.tensor_add(out=mind2, in0=rT, in1=P2)
        nc.vector.tensor_scalar(out=mind2, in0=mind2, scalar1=0.0, scalar2=None, op0=AL.max)

        stack2 = temps.tile([64, 2], F32)
        masked = temps.tile([64, RES], F32)
        nc.vector.scalar_tensor_tensor(
            out=masked, in0=mind2, scalar=1.0, in1=mask,
            op0=AL.mult, op1=AL.mult,
            accum_out=stack2[:, 0:1],
        )
        nc.vector.tensor_reduce(
            out=stack2[:, 1:2], in_=mask, axis=mybir.AxisListType.X, op=AL.add
        )

        out_ps = psum.tile([1, 2], F32, tag="psDB0")
        nc.tensor.matmul(out_ps, lhsT=ones64, rhs=stack2, start=True, stop=True)
        out_sb = temps.tile([1, 2], F32)
        nc.vector.tensor_copy(out=out_sb, in_=out_ps)
        nc.sync.dma_start(out=out_d[:, :], in_=out_sb)


def _get_nc():
    global _CACHED_NC
    if _CACHED_NC is None:
        _CACHED_NC = _build_nc()
    return _CACHED_NC


def _np8():
    return np.dtype(mybir.dt.np(FP8))


def _split3_fp8(x, np8):
    """3-level fp8 decomposition: h + l + m ~= x (each rounded RNE)."""
    x = x.astype(np.float32)
    h = x.astype(np8)
    r1 = x - h.astype(np.float32)
    l = r1.astype(np8)
    r2 = r1 - l.astype(np.float32)
    m = r2.astype(np8)
    return h, l, m


def _host_rays(c_row, half, depth_half):
    """Exact float64 mirror of the reference ray sampler for this half's
    8192 rays; returns pred [8192,3] float64 and |pred|^2 float32."""
    c64 = c_row.astype(np.float64)
    cam2world = c64[:16].reshape(4, 4)
    intr = c64[16:25].reshape(3, 3)
    fx, fy = intr[0, 0], intr[1, 1]
    cx, cy, sk = intr[0, 2], intr[1, 2], intr[0, 1]
    R = cam2world[:3, :3]
    t = cam2world[:3, 3]
    m = np.arange(half * MLOC, (half + 1) * MLOC)
    ii = (m // RES).astype(np.float64)   # row -> y
    jj = (m % RES).astype(np.float64)    # col -> x
    x = (jj + 0.5) / RES
    y = (ii + 0.5) / RES
    x_lift = (x - cx + cy * sk / fy - sk * y / fy) / fx
    y_lift = (y - cy) / fy
    cam_rel = np.stack([x_lift, y_lift, np.ones_like(x)], axis=-1)  # [MLOC,3]
    dirs = cam_rel @ R.T
    dirs = dirs / np.maximum(np.linalg.norm(dirs, axis=-1, keepdims=True), 1e-12)
    pred = t[None, :] + depth_half.astype(np.float64)[:, None] * dirs
    p2 = (pred * pred).sum(-1).astype(np.float32)
    return pred, p2


def _host_lrows(pred):
    """L-side fp8 rows [12, 2, 8192] from pred [8192,3].  Kind list must
    pair with _host_rrows:
      per coord c: Ph Ph Pl Pl Ph Pm; extra z: Pl Pm; ones x3; zero."""
    np8 = _np8()
    kinds = []
    for c in range(3):
        Ph, Pl, Pm = _split3_fp8(pred[:, c].astype(np.float32), np8)
        kinds += [Ph, Ph, Pl, Pl, Ph, Pm]
        if c == 2:
            kinds += [Pl, Pm]
    ones = np.ones(MLOC, np8)
    kinds += [ones, ones, ones]
    kinds.append(np.zeros(MLOC, np8))
    assert len(kinds) == NROWS
    out = np.zeros((NP8, 2, MLOC), np8)
    for k, vals in enumerate(kinds):
        out[k // 2, k % 2, :] = vals
    return out.reshape(-1)


def _host_rrows(pc_b):
    """R-side fp8 rows [12, 2, 2048] for one batch: pair sums/diffs.

    Columns 0:1024 are s-pairs (a+b), 1024:2048 d-pairs (a-b).  Kind list:
      per coord c: (Ph,Vh) (Ph,Vl) (Pl,Vh) (Pl,Vl) (Ph,Vm) (Pm,Vh)
      extra z terms: (Pl,Vm) (Pm,Vl)
      u rows: (1,Uh) (1,Ul) (1,Um); zero pad row.
    """
    np8 = _np8()
    pc64 = pc_b.astype(np.float64)
    a = pc64[0::2]   # [1024, 3]
    b = pc64[1::2]
    vs = -2.0 * (a + b)
    vd = -2.0 * (a - b)
    us = (a * a).sum(-1) + (b * b).sum(-1)
    ud = (a * a).sum(-1) - (b * b).sum(-1)
    kinds = []
    for c in range(3):
        v = np.concatenate([vs[:, c], vd[:, c]]).astype(np.float32)
        Vh, Vl, Vm = _split3_fp8(v, np8)
        kinds += [Vh, Vl, Vh, Vl, Vm, Vh]
        if c == 2:
            kinds += [Vm, Vl]
    u = np.concatenate([us, ud]).astype(np.float32)
    Uh, Ul, Um = _split3_fp8(u, np8)
    kinds += [Uh, Ul, Um]
    kinds.append(np.zeros(N, np8))
    assert len(kinds) == NROWS
    out = np.zeros((NP8, 2, N), np8)
    for k, vals in enumerate(kinds):
        out[k // 2, k % 2, :] = vals.astype(np8)
    return out.reshape(-1)


def _make_in_maps(c, image_depth, pc):
    in_maps = []
    rrows = [_host_rrows(pc[b]) for b in range(B)]
    mds = [
        float(np.sqrt(((c[b, :16].reshape(4, 4)[:3, 3].astype(np.float64)[None, :]
                        - pc[b].astype(np.float64)) ** 2).sum(-1).max()))
        for b in range(B)
    ]
    for core in range(8):
        b, h = core // 2, core % 2
        depth_half = np.ascontiguousarray(
            image_depth[b].reshape(M)[h * MLOC : (h + 1) * MLOC]
        ).astype(np.float32)
        pred, p2 = _host_rays(np.asarray(c[b]), h, depth_half)
        par = np.zeros(NPAR, np.float32)
        par[P_MD] = mds[b]
        in_maps.append(
            {
                "depth": depth_half,
                "rin": rrows[b],
                "lin": _host_lrows(pred),
                "p2in": p2,
                "params": par,
            }
        )
    return in_maps


def _install_ntff_hook():
    """antenv.axon_hooks is missing on this image; inject an equivalent so
    trace=True can capture NTFF profiles through libaxon_pjrt.so."""
    import types

    if "antenv.axon_hooks" in sys.modules:
        return
    mod = types.ModuleType("antenv.axon_hooks")
    holder = [None]
    mod.set_axon_ntff_profile_hook = lambda h: holder.__setitem__(0, h)
    mod.get_axon_ntff_profile_hook = lambda: holder[0]
    sys.modules["antenv.axon_hooks"] = mod
    try:
        import antenv

        antenv.axon_hooks = mod
    except ImportError:
        pass
    try:
        from trn_agent_boot.trn_boot import _ntff_profile_via_ctypes

        mod.set_axon_ntff_profile_hook(
            _ntff_profile_via_ctypes("/opt/axon/libaxon_pjrt.so")
        )
    except Exception:
        pass


def run(c, image_depth, pc, trace=False):
    from concourse.bass_utils import run_bass_kernel_spmd

    if trace:
        _install_ntff_hook()

    nc = _get_nc()
    in_maps = _make_in_maps(np.asarray(c), np.asarray(image_depth), np.asarray(pc))
    res = run_bass_kernel_spmd(nc, in_maps, core_ids=list(range(8)), trace=trace)
    loss = np.zeros((B, 1), np.float32)
    for b in range(B):
        v0 = res.results[2 * b]["out"].ravel()
        v1 = res.results[2 * b + 1]["out"].ravel()
        num = v0[0] + v1[0]
        den = v0[1] + v1[1]
        loss[b, 0] = num / max(den, 1.0)
    return loss, res


def kernel(c, image_depth, pc, neural_rendering_resolution):
    assert int(neural_rendering_resolution) == RES
    loss, _ = run(c, image_depth, pc, trace=False)
    return loss


# revision 31
# speedup vs baseline: 1.0106x; 1.0106x over previous
"""Chamfer-distance block (EG3D ray sampler + point-cloud chamfer) on 8 trn2 cores.

Sharding: core c = 2*b + h handles batch b (of 4) and ray-half h (8192 of
16384 rays).  Host does the O(M)+O(N) prep (exact float64 ray sampler,
pred points, fp8 3-level hi/lo splits, point-pair transform, max_distance);
the device does the O(M*N) retrieval:
  - row-min over the 8192x2048 squared-distance matrix via the min-pair
    identity min(a,b) = (a+b - |a-b|)/2: the PE emits pair-sum columns
    s_j = cross_2j + cross_2j+1 and pair-diff columns d_j = cross_2j -
    cross_2j+1 (1024 of each per 128-ray tile) using fp8e4m3 3-level
    hi/lo-split matmuls in DoubleRow perf mode (24 k-rows as 12
    partitions x 2 groups).  Drain per tile: ACT reads the d-half of
    PSUM (Abs -> fp16 SBUF), DVE reads the s-half with one fused custom
    DVE op ((s - |d|)*0.5, min-accumulated per ray), so each engine
    streams exactly half the PSUM traffic.  The d-halves are split into
    single-bank PSUM tiles with their own ABS each, so banks free early
    and the PE never stalls at tile-pair boundaries (a stall resets the
    PE's p-state clock ramp and slows every matmul 2-4x).
  - masked partial sums (numerator, denominator)
Host combines the two halves of each batch: loss = num / max(den, 1).
"""

import os
import sys

import numpy as np

if "/opt/trn_rl_repo" not in sys.path:
    sys.path.insert(0, "/opt/trn_rl_repo")

import concourse.bass as bass
import concourse.bacc as bacc
import concourse.mybir as mybir
import concourse.tile as tile
from concourse.bass import ts
from concourse.masks import make_identity

F32 = mybir.dt.float32
F16 = mybir.dt.float16
FP8 = mybir.dt.float8e4
I32 = mybir.dt.int32

B = 4
RES = 128
M = RES * RES          # 16384 rays per batch
MLOC = M // 2          # 8192 rays per core
N = 2048               # points
NPAIR = N // 2         # 1024 point pairs
NT = 64                # m-tiles of 128 rays per core
NPAR = 16              # host-computed per-core scalar params
NROWS = 24             # fp8 k-rows (12 partitions x 2 DoubleRow groups)
NP8 = NROWS // 2

P_MD = 0               # params[0] = max_distance

_CACHED_NC = None
KVER = 37  # bump to bust the NEFF cache when kernel structure changes


def _register_pair_min_op():
    """Runtime-register a custom DVE op: out=(in0-in1)*imm2 with a
    min-reduce accumulator seeded from s0.  One DVE pass fuses the
    (s - |d|)*0.5 pair-min with the per-ray min reduction (the stock
    InstTensorTensorReduce crashes the exec unit on this fw)."""
    import concourse.dve_ops as dve_ops

    name = "PAIR_MIN_REDUCE_ANT"
    for op in dve_ops.OPS:
        if op.name == name:
            return op
    from concourse.dve_spec import C0, C2, Spec, Src0, Src1, _has_src1, lower, minn
    from concourse.dve_uop import DveOpSpec

    def _ref(in0, in1, c0, c1, c2):
        body = (in0.astype(np.float32) - in1) * c2
        return body, dve_ops._accum_ref(body, c0, minn, False)

    spec = Spec(body=(Src0 - Src1) * C2, accum=minn, accum_init=C0, reference=_ref)
    row = max(dve_ops._SUB_OPCODE_FOR_NAME.values()) + 1
    assert row < 0x20
    dve_ops._SUB_OPCODE_FOR_NAME[name] = row
    shas = {}
    for ver in ("v3", "v4"):
        shas[ver] = DveOpSpec(
            name=name, opcode=row, uops=lower(spec, ver=ver), rd1_en=_has_src1(spec)
        ).sha(ver)
    op = dve_ops.DveOp(name, spec, subdim=False, uops_sha=shas)
    dve_ops.OPS.append(op)
    dve_ops.CUSTOM_DVE_SPECS[name] = spec
    return op


def _patch_compiler_flags():
    """Enable walrus's ldweights dedup: consecutive matmuls sharing a
    stationary operand skip the redundant reload (PE is LDW-serialized
    otherwise)."""
    from concourse import bass_utils as _bu

    if getattr(_bu, "_ldwopt_patched", False):
        return
    _orig = _bu.run_command

    def _patched(argv, **kw):
        return _orig(argv, **kw)

    _bu.run_command = _patched
    _bu._ldwopt_patched = True


def _build_nc():
    _patch_compiler_flags()
    nc = bacc.Bacc()
    nc.dram_tensor(f"ver{KVER}", [1], F32)
    depth_d = nc.dram_tensor("depth", [MLOC], F32, kind="ExternalInput")
    rin_d = nc.dram_tensor("rin", [NP8 * 2 * N], FP8, kind="ExternalInput")
    lin_d = nc.dram_tensor("lin", [NP8 * 2 * MLOC], FP8, kind="ExternalInput")
    p2_d = nc.dram_tensor("p2in", [MLOC], F32, kind="ExternalInput")
    par_d = nc.dram_tensor("params", [NPAR], F32, kind="ExternalInput")
    out_d = nc.dram_tensor("out", [1, 2], F32, kind="ExternalOutput")

    with tile.TileContext(nc) as tc:
        _trace_kernel(tc, depth_d, rin_d, lin_d, p2_d, par_d, out_d)
    nc.finalize()
    return nc


def _trace_kernel(tc, depth_d, rin_d, lin_d, p2_d, par_d, out_d):
    nc = tc.nc
    AL = mybir.AluOpType
    ACT = mybir.ActivationFunctionType
    pair_min_op = _register_pair_min_op()

    import contextlib

    with contextlib.ExitStack() as ctx:
        singles = ctx.enter_context(tc.tile_pool(name="singles", bufs=1))
        temps = ctx.enter_context(tc.tile_pool(name="temps", bufs=2))
        psum = ctx.enter_context(tc.tile_pool(name="psum", bufs=1, space="PSUM"))
        scratchp = ctx.enter_context(tc.tile_pool(name="scratchp", bufs=4))

        # ---- load inputs (all DMAs fan out in parallel) ------------------
        par = singles.tile([128, NPAR], F32)
        nc.sync.dma_start(
            out=par,
            in_=bass.AP(tensor=par_d, offset=0, ap=[[0, 128], [1, NPAR]]),
        )

        D = singles.tile([64, RES], F32)  # depth, ray m = p*128 + f
        nc.sync.dma_start(out=D, in_=depth_d.rearrange("(p f) -> p f", f=RES))

        P2 = singles.tile([64, RES], F32)  # |pred|^2, host-computed
        nc.sync.dma_start(out=P2, in_=p2_d.rearrange("(p f) -> p f", f=RES))

        # R-side fp8 rows (host): [12, 2 groups, 2048] dup'd per quadrant.
        # Separate A/B tiles so each quadrant's matmuls start as soon as its
        # own DMAs land.
        RbufA = singles.tile([128, 2 * N], FP8)
        RbufB = singles.tile([128, 2 * N], FP8)
        qeng = [nc.sync, nc.gpsimd, nc.scalar]
        rv_in = rin_d.rearrange("(p g f) -> p g f", g=2, f=N)
        for qi_r, (rb, base) in enumerate(((RbufA, 0), (RbufB, 64))):
            qeng[qi_r].dma_start(
                out=rb[base : base + NP8, :].rearrange("p (g f) -> p g f", g=2),
                in_=rv_in,
            )

        # L-side fp8 rows (host): [12, 2, 8192]; m-tiles 0-31 -> quadrant A,
        # 32-63 -> B.  Chunked along rays for DMA-queue overlap.
        LbufA = singles.tile([128, 2 * 4096], FP8)
        LbufB = singles.tile([128, 2 * 4096], FP8)
        lv_in = lin_d.rearrange("(p g f) -> p g f", g=2, f=MLOC)
        qi = 0
        CH = 1024
        for lb, base, lo in ((LbufA, 0, 0), (LbufB, 64, 4096)):
            for c0 in range(0, 4096, CH):
                qeng[(2 + qi) % 3].dma_start(
                    out=lb[base : base + NP8, :].rearrange("p (g f) -> p g f", g=2)[
                        :, :, c0 : c0 + CH
                    ],
                    in_=lv_in[:, :, lo + c0 : lo + c0 + CH],
                )
                qi += 1

        identity = singles.tile([128, 128], F32)
        make_identity(nc, identity)

        # warm the ACT Abs table before the loop
        warm = singles.tile([1, 1], F32)
        nc.vector.memset(warm, 1.0)
        nc.scalar.activation(out=warm, in_=warm, func=ACT.Abs, bias=0.0, scale=1.0)

        # ray mask: depth < max_distance (host-computed scalar)
        mask = temps.tile([64, RES], F32)
        nc.vector.tensor_scalar(
            out=mask, in0=D, scalar1=par[:64, P_MD : P_MD + 1], scalar2=None, op0=AL.is_lt
        )
        ones64 = singles.tile([64, 1], F32)
        nc.vector.memset(ones64, 1.0)

        # ---- main loop: DoubleRow matmuls + split s/|d| drains -----------
        # d-chunks are emitted before s-chunks so each tile's psD frees
        # early (ABS starts while the s-matmuls still stream).
        rmin = singles.tile([128, NT], F32)
        DR = mybir.MatmulPerfMode.DoubleRow

        LvA = LbufA.rearrange("p (g c) -> p g c", g=2)
        LvB = LbufB.rearrange("p (g c) -> p g c", g=2)
        RvA = RbufA.rearrange("p (g c) -> p g c", g=2)
        RvB = RbufB.rearrange("p (g c) -> p g c", g=2)

        for p in range(NT // 2):
            psS_A = psum.tile([128, NPAIR], F32, tag="psSA")
            psS_B = psum.tile([128, NPAIR], F32, tag="psSB")
            psD_A0 = psum.tile([128, 512], F32, tag="psDA0")
            psD_A1 = psum.tile([128, 512], F32, tag="psDA1")
            psD_B0 = psum.tile([128, 512], F32, tag="psDB0")
            psD_B1 = psum.tile([128, 512], F32, tag="psDB1")
            for quad, nt in (
                ("A", 2), ("A", 3), ("B", 2), ("B", 3),
                ("A", 0), ("B", 0), ("A", 1), ("B", 1),
            ):
                off = (nt % 2) * 512
                if quad == "A":
                    if nt == 2:
                        dst, doff = psD_A0, 0
                    elif nt == 3:
                        dst, doff = psD_A1, 0
                    else:
                        dst, doff = psS_A, off
                    nc.tensor.matmul(
                        dst[:, doff : doff + 512],
                        lhsT=LvA[0:NP8, :, ts(p, 128)],
                        rhs=RvA[0:NP8, :, ts(nt, 512)],
                        start=True, stop=True, tile_position=(0, 0),
                        perf_mode=DR,
                    )
                else:
                    if nt == 2:
                        dst, boff = psD_B0, 0
                    elif nt == 3:
                        dst, boff = psD_B1, 0
                    else:
                        dst, boff = psS_B, off
                    nc.tensor.matmul(
                        dst[:, boff : boff + 512],
                        lhsT=LvB[64 : 64 + NP8, :, ts(p, 128)],
                        rhs=RvB[64 : 64 + NP8, :, ts(nt, 512)],
                        start=True, stop=True, tile_position=(64, 0),
                        perf_mode=DR,
                    )
            absd_A = scratchp.tile([128, NPAIR], F16, tag="absdA")
            nc.scalar.activation(out=absd_A[:, 0:512], in_=psD_A0, func=ACT.Abs, bias=0.0, scale=1.0)
            nc.scalar.activation(out=absd_A[:, 512:1024], in_=psD_A1, func=ACT.Abs, bias=0.0, scale=1.0)
            absd_B = scratchp.tile([128, NPAIR], F16, tag="absdB")
            nc.scalar.activation(out=absd_B[:, 0:512], in_=psD_B0, func=ACT.Abs, bias=0.0, scale=1.0)
            nc.scalar.activation(out=absd_B[:, 512:1024], in_=psD_B1, func=ACT.Abs, bias=0.0, scale=1.0)
            for ps_s, absd, t in ((psS_A, absd_A, p), (psS_B, absd_B, 32 + p)):
                junk = scratchp.tile([128, NPAIR], F16, tag="junk")
                nc.vector._custom_dve(
                    pair_min_op, out=junk, in0=ps_s[:, :], in1=absd[:, :],
                    s0=1e30, s1=0.0, imm2=0.5,
                    accum_out=rmin[:, t : t + 1],
                )

        # ---- final: transpose rmin back to ray layout, mask, sums --------
        rT = psum.tile([64, 128], F32, tag="psSA")
        nc.tensor.transpose(rT, rmin, identity)

        mind2 = temps.tile([64, RES], F32)
        nc.vector.tensor_add(out=mind2, in0=rT, in1=P2)
        nc.vector.tensor_scalar(out=mind2, in0=mind2, scalar1=0.0, scalar2=None, op0=AL.max)

        stack2 = temps.tile([64, 2], F32)
        masked = temps.tile([64, RES], F32)
        nc.vector.scalar_tensor_tensor(
            out=masked, in0=mind2, scalar=1.0, in1=mask,
            op0=AL.mult, op1=AL.mult,
            accum_out=stack2[:, 0:1],
        )
        nc.vector.tensor_reduce(
            out=stack2[:, 1:2], in_=mask, axis=mybir.AxisListType.X, op=AL.add
        )

        out_ps = psum.tile([1, 2], F32, tag="psDB0")
        nc.tensor.matmul(out_ps, lhsT=ones64, rhs=stack2, start=True, stop=True)
        out_sb = temps.tile([1, 2], F32)
        nc.vector.tensor_copy(out=out_sb, in_=out_ps)
        nc.sync.dma_start(out=out_d[:, :], in_=out_sb)


def _get_nc():
    global _CACHED_NC
    if _CACHED_NC is None:
        _CACHED_NC = _build_nc()
    return _CACHED_NC


def _np8():
    return np.dtype(mybir.dt.np(FP8))


def _split3_fp8(x, np8):
    """3-level fp8 decomposition: h + l + m ~= x (each rounded RNE)."""
    x = x.astype(np.float32)
    h = x.astype(np8)
    r1 = x - h.astype(np.float32)
    l = r1.astype(np8)
    r2 = r1 - l.astype(np.float32)
    m = r2.astype(np8)
    return h, l, m


def _host_rays(c_row, half, depth_half):
    """Exact float64 mirror of the reference ray sampler for this half's
    8192 rays; returns pred [8192,3] float64 and |pred|^2 float32."""
    c64 = c_row.astype(np.float64)
    cam2world = c64[:16].reshape(4, 4)
    intr = c64[16:25].reshape(3, 3)
    fx, fy = intr[0, 0], intr[1, 1]
    cx, cy, sk = intr[0, 2], intr[1, 2], intr[0, 1]
    R = cam2world[:3, :3]
    t = cam2world[:3, 3]
    m = np.arange(half * MLOC, (half + 1) * MLOC)
    ii = (m // RES).astype(np.float64)   # row -> y
    jj = (m % RES).astype(np.float64)    # col -> x
    x = (jj + 0.5) / RES
    y = (ii + 0.5) / RES
    x_lift = (x - cx + cy * sk / fy - sk * y / fy) / fx
    y_lift = (y - cy) / fy
    cam_rel = np.stack([x_lift, y_lift, np.ones_like(x)], axis=-1)  # [MLOC,3]
    dirs = cam_rel @ R.T
    dirs = dirs / np.maximum(np.linalg.norm(dirs, axis=-1, keepdims=True), 1e-12)
    pred = t[None, :] + depth_half.astype(np.float64)[:, None] * dirs
    p2 = (pred * pred).sum(-1).astype(np.float32)
    return pred, p2


def _host_lrows(pred):
    """L-side fp8 rows [12, 2, 8192] from pred [8192,3].  Kind list must
    pair with _host_rrows:
      per coord c: Ph Ph Pl Pl Ph Pm; extra z: Pl Pm; ones x3; zero."""
    np8 = _np8()
    kinds = []
    for c in range(3):
        Ph, Pl, Pm = _split3_fp8(pred[:, c].astype(np.float32), np8)
        kinds += [Ph, Ph, Pl, Pl, Ph, Pm]
        if c == 2:
            kinds += [Pl, Pm]
    ones = np.ones(MLOC, np8)
    kinds += [ones, ones, ones]
    kinds.append(np.zeros(MLOC, np8))
    assert len(kinds) == NROWS
    out = np.zeros((NP8, 2, MLOC), np8)
    for k, vals in enumerate(kinds):
        out[k // 2, k % 2, :] = vals
    return out.reshape(-1)


def _host_rrows(pc_b):
    """R-side fp8 rows [12, 2, 2048] for one batch: pair sums/diffs.

    Columns 0:1024 are s-pairs (a+b), 1024:2048 d-pairs (a-b).  Kind list:
      per coord c: (Ph,Vh) (Ph,Vl) (Pl,Vh) (Pl,Vl) (Ph,Vm) (Pm,Vh)
      extra z terms: (Pl,Vm) (Pm,Vl)
      u rows: (1,Uh) (1,Ul) (1,Um); zero pad row.
    """
    np8 = _np8()
    pc64 = pc_b.astype(np.float64)
    a = pc64[0::2]   # [1024, 3]
    b = pc64[1::2]
    vs = -2.0 * (a + b)
    vd = -2.0 * (a - b)
    us = (a * a).sum(-1) + (b * b).sum(-1)
    ud = (a * a).sum(-1) - (b * b).sum(-1)
    kinds = []
    for c in range(3):
        v = np.concatenate([vs[:, c], vd[:, c]]).astype(np.float32)
        Vh, Vl, Vm = _split3_fp8(v, np8)
        kinds += [Vh, Vl, Vh, Vl, Vm, Vh]
        if c == 2:
            kinds += [Vm, Vl]
    u = np.concatenate([us, ud]).astype(np.float32)
    Uh, Ul, Um = _split3_fp8(u, np8)
    kinds += [Uh, Ul, Um]
    kinds.append(np.zeros(N, np8))
    assert len(kinds) == NROWS
    out = np.zeros((NP8, 2, N), np8)
    for k, vals in enumerate(kinds):
        out[k // 2, k % 2, :] = vals.astype(np8)
    return out.reshape(-1)


def _make_in_maps(c, image_depth, pc):
    in_maps = []
    rrows = [_host_rrows(pc[b]) for b in range(B)]
    mds = [
        float(np.sqrt(((c[b, :16].reshape(4, 4)[:3, 3].astype(np.float64)[None, :]
                        - pc[b].astype(np.float64)) ** 2).sum(-1).max()))
        for b in range(B)
    ]
    for core in range(8):
        b, h = core // 2, core % 2
        depth_half = np.ascontiguousarray(
            image_depth[b].reshape(M)[h * MLOC : (h + 1) * MLOC]
        ).astype(np.float32)
        pred, p2 = _host_rays(np.asarray(c[b]), h, depth_half)
        par = np.zeros(NPAR, np.float32)
        par[P_MD] = mds[b]
        in_maps.append(
            {
                "depth": depth_half,
                "rin": rrows[b],
                "lin": _host_lrows(pred),
                "p2in": p2,
                "params": par,
            }
        )
    return in_maps


def _install_ntff_hook():
    """antenv.axon_hooks is missing on this image; inject an equivalent so
    trace=True can capture NTFF profiles through libaxon_pjrt.so."""
    import types

    if "antenv.axon_hooks" in sys.modules:
        return
    mod = types.ModuleType("antenv.axon_hooks")
    holder = [None]
    mod.set_axon_ntff_profile_hook = lambda h: holder.__setitem__(0, h)
    mod.get_axon_ntff_profile_hook = lambda: holder[0]
    sys.modules["antenv.axon_hooks"] = mod
    try:
        import antenv

        antenv.axon_hooks = mod
    except ImportError:
        pass
    try:
        from trn_agent_boot.trn_boot import _ntff_profile_via_ctypes

        mod.set_axon_ntff_profile_hook(
            _ntff_profile_via_ctypes("/opt/axon/libaxon_pjrt.so")
        )
    except Exception:
        pass


def run(c, image_depth, pc, trace=False):
    from concourse.bass_utils import run_bass_kernel_spmd

    if trace:
        _install_ntff_hook()

    nc = _get_nc()
    in_maps = _make_in_maps(np.asarray(c), np.asarray(image_depth), np.asarray(pc))
    res = run_bass_kernel_spmd(nc, in_maps, core_ids=list(range(8)), trace=trace)
    loss = np.zeros((B, 1), np.float32)
    for b in range(B):
        v0 = res.results[2 * b]["out"].ravel()
        v1 = res.results[2 * b + 1]["out"].ravel()
        num = v0[0] + v1[0]
        den = v0[1] + v1[1]
        loss[b, 0] = num / max(den, 1.0)
    return loss, res


def kernel(c, image_depth, pc, neural_rendering_resolution):
    assert int(neural_rendering_resolution) == RES
    loss, _ = run(c, image_depth, pc, trace=False)
    return loss
